# revision 35
# baseline (speedup 1.0000x reference)
"""Trainium2 Bass kernel for nn_AJSSMamba (adaptive directional scan).

Self-contained: shards batch 8 across 8 NeuronCores (1 sample/core),
computes the full module on-device, gathers outputs on host.

Per-core pipeline (sample x [96,256,256] f32):
  1. channel mean (streamed)         -> xg [256,256]
  2. sobel complexity + minmax norm  -> m
  3. local 3x3 avg -> adj; quadrant means -> base; steps[d] = clip(base_d+adj,1,5)
  4. four directional adaptive-step traversals via a block-automaton scan:
     256 positions = 16 blocks x 16; 5 entry offsets packed as base-2 bit
     planes of one bf16 number (out-degree-1 traversal => no carries);
     cross-block chaining via 5-state one-hot automaton.
  5. V = sum of 4 visit masks; factor = V/(V+1e-6); out = x * factor.
"""

import sys

sys.path.insert(0, "/opt/trn_rl_repo")

import math

import numpy as np

P = 128          # partitions
C = 96           # channels
H = 256
W = 256
A = 2            # H // P
CC = 8           # channels per stream chunk
NCHUNK = C // CC
G = 16           # scan block size
NB = 16          # number of blocks (G*NB == W)
RG = 8           # row groups: 4 directions x (256 rows / 128)
RT = RG * NB     # collapsed (rg, block) groups

LN3 = math.log(3.0)
ADJ_HI = 9.0 * (2.5 + LN3) / 5.0   # lc9 threshold for adj=+1
ADJ_LO = 9.0 * (2.5 - LN3) / 5.0   # lc9 threshold for adj=-1
# base = 1 + sum_k [pre < thr_k], pre = qsum*(5/16384) - 2.5
BASE_THR = [math.log(7.0), math.log(5.0 / 3.0), -math.log(5.0 / 3.0), -math.log(7.0)]

_NC_CACHE = {}


def _build_nc():
    from concourse import bacc, mybir
    from concourse.tile import TileContext
    from concourse import bass_isa

    f32 = mybir.dt.float32
    bf16 = mybir.dt.bfloat16
    i32 = mybir.dt.int32
    Alu = mybir.AluOpType
    Act = mybir.ActivationFunctionType

    nc = bacc.Bacc(None, target_bir_lowering=False, debug=False)
    x = nc.declare_dram_parameter("x", [C, H, W], f32, isOutput=False)
    out = nc.declare_dram_parameter("out", [C, H, W], f32, isOutput=True)

    with TileContext(nc) as tc:
        with (
            tc.tile_pool(name="stream", bufs=5) as stream,
            tc.tile_pool(name="maps", bufs=1) as maps,
            tc.tile_pool(name="scan", bufs=1) as scan,
            tc.tile_pool(name="smalls", bufs=2) as smalls,
            tc.tile_pool(name="consts", bufs=1) as consts,
            tc.tile_pool(name="psum", bufs=4, space="PSUM") as psum,
        ):
            # ---------------- constants (banded matrices for vertical
            # filters along h = 2p+i, and the PE-transpose identity) -----
            ITi = consts.tile([P, 128], i32, tag="ITi")
            nc.gpsimd.iota(ITi[:, :], pattern=[[-1, 128]], base=0,
                           channel_multiplier=1)  # value = k - m
            D0f = consts.tile([P, 128], f32, tag="D0f")
            Dm1 = consts.tile([P, 128], f32, tag="Dm1")
            Dp1 = consts.tile([P, 128], f32, tag="Dp1")
            D2f = consts.tile([P, 128], f32, tag="D2f")
            A01 = consts.tile([P, 128], f32, tag="A01")  # D0 + Dm1
            A10 = consts.tile([P, 128], f32, tag="A10")  # D0 + Dp1
            M01 = consts.tile([P, 128], f32, tag="M01")  # Dm1 - D0
            M10 = consts.tile([P, 128], f32, tag="M10")  # D0 - Dp1
            IDb = consts.tile([P, 128], bf16, tag="IDb")
            nc.vector.tensor_scalar(D0f[:, :], ITi[:, :], 0.0, None, Alu.is_equal)
            nc.vector.tensor_scalar(Dm1[:, :], ITi[:, :], -1.0, None, Alu.is_equal)
            nc.vector.tensor_scalar(Dp1[:, :], ITi[:, :], 1.0, None, Alu.is_equal)
            nc.vector.tensor_scalar_mul(D2f[:, :], D0f[:, :], 2.0)
            nc.vector.tensor_add(A01[:, :], D0f[:, :], Dm1[:, :])
            nc.vector.tensor_add(A10[:, :], D0f[:, :], Dp1[:, :])
            nc.vector.tensor_sub(M01[:, :], Dm1[:, :], D0f[:, :])
            nc.vector.tensor_sub(M10[:, :], D0f[:, :], Dp1[:, :])
            nc.vector.tensor_scalar(IDb[:, :], ITi[:, :], 0.0, None, Alu.is_equal)

            def vfilter(dst, src, lhs0_for_i0, lhs1_for_i0, lhs0_for_i1,
                        lhs1_for_i1):
                # dst[:, i, :] = banded vertical filter of src (padded W+2)
                for i in range(2):
                    l0 = lhs0_for_i0 if i == 0 else lhs0_for_i1
                    l1 = lhs1_for_i0 if i == 0 else lhs1_for_i1
                    ps = psum.tile([P, W + 2], f32, tag="ps")
                    if l0 is not None and l1 is not None:
                        nc.tensor.matmul(ps[:, :], l0, src[:, 0, :],
                                         start=True, stop=False)
                        nc.tensor.matmul(ps[:, :], l1, src[:, 1, :],
                                         start=False, stop=True)
                    elif l0 is not None:
                        nc.tensor.matmul(ps[:, :], l0, src[:, 0, :],
                                         start=True, stop=True)
                    else:
                        nc.tensor.matmul(ps[:, :], l1, src[:, 1, :],
                                         start=True, stop=True)
                    nc.scalar.copy(dst[:, i, :], ps[:, :])

            def pe_transpose(dst, src):
                # dst = src.T for [128,128] bf16 tiles via PE
                tp = psum.tile([P, 128], bf16, tag="tp")
                nc.tensor.transpose(tp[:, :], src, IDb[:, :])
                nc.scalar.copy(dst, tp[:, :])

            # NV[h,w] = number of in-bounds 3x3 neighbors (9/6/4) -- used to
            # fold the min-max normalization into scalar thresholds
            ONESP = consts.tile([P, A, W + 2], f32, tag="ONESP")
            nc.vector.memset(ONESP[:, :, :], 0.0)
            nc.vector.memset(ONESP[:, :, 1:W + 1], 1.0)
            XV = consts.tile([P, A, W + 2], f32, tag="XV")
            vfilter(XV, ONESP, D0f[:, :], A01[:, :], A10[:, :], D0f[:, :])
            NV = consts.tile([P, A, W], f32, tag="NV")
            nc.vector.tensor_add(NV[:, :, :], XV[:, :, 0:W], XV[:, :, 1:W + 1])
            nc.vector.tensor_add(NV[:, :, :], NV[:, :, :], XV[:, :, 2:W + 2])
            # ---------------- pass 1: channel mean ----------------
            acc = maps.tile([P, A, W], f32, tag="acc")
            acc2 = maps.tile([P, A, W], f32, tag="acc2")
            for ci in range(NCHUNK):
                xt = stream.tile([P, CC, A, W], f32, tag="xin")
                ldeng = nc.sync if ci % 2 == 0 else nc.scalar
                ldeng.dma_start(
                    out=xt[:, :, :, :],
                    in_=x[ci * CC:(ci + 1) * CC].rearrange(
                        "c (p i) w -> p c i w", i=2
                    ),
                )
                # tree-reduce the 8 channels: 8 -> 4 -> 2 -> 1
                # alternate chunks between DVE and GpSimd (both 1x fp32);
                # per-engine partial accumulators merge at the end.
                on_gps = (ci % 3 == 2)
                eng = nc.gpsimd if on_gps else nc.vector
                accx = acc2 if on_gps else acc
                first = (ci == 2) if on_gps else (ci == 0)
                if on_gps:
                    t4 = scan.tile([P, 4, A, W], f32, tag="VP")
                    t2 = scan.tile([P, 2, A, W], f32, tag="vis")
                else:
                    t4 = scan.tile([P, 4, A, W], f32, tag="Rt")
                    t2 = scan.tile([P, 2, A, W], f32, tag="SS")
                eng.tensor_add(t4[:, :, :, :], xt[:, 0:4, :, :], xt[:, 4:8, :, :])
                eng.tensor_add(t2[:, :, :, :], t4[:, 0:2, :, :], t4[:, 2:4, :, :])
                if first:
                    eng.tensor_add(accx[:, :, :], t2[:, 0, :, :], t2[:, 1, :, :])
                else:
                    eng.tensor_add(t2[:, 0, :, :], t2[:, 0, :, :], t2[:, 1, :, :])
                    eng.tensor_add(accx[:, :, :], accx[:, :, :], t2[:, 0, :, :])
            nc.vector.tensor_add(acc[:, :, :], acc[:, :, :], acc2[:, :, :])

            # xg = acc / 96, into W-padded buffer MP [P, A, W+2]
            MP = maps.tile([P, A, W + 2], f32, tag="MP")
            nc.vector.memset(MP[:, :, :], 0.0)
            nc.vector.tensor_scalar_mul(MP[:, :, 1:W + 1], acc[:, :, :], 1.0 / C)

            # ---------------- sobel ----------------
            # vertical filters along h via banded matmuls on the (idle) PE:
            # X1 = [1,2,1]_v xg (padded), X2 = [1,0,-1]_v xg
            X1 = maps.tile([P, A, W + 2], f32, tag="X1")
            X2 = maps.tile([P, A, W + 2], f32, tag="X2")
            vfilter(X1, MP, D2f[:, :], A01[:, :], A10[:, :], D2f[:, :])
            vfilter(X2, MP, None, M01[:, :], M10[:, :], None)

            # horizontal: gx = X1[w-1]-X1[w+1]; gy = X2[w-1]+2*X2[w]+X2[w+1]
            gx = maps.tile([P, A, W], f32, tag="gx")
            gy = maps.tile([P, A, W], f32, tag="gy")
            nc.vector.tensor_sub(gx[:, :, :], X1[:, :, 0:W], X1[:, :, 2:W + 2])
            nc.vector.scalar_tensor_tensor(
                gy[:, :, :], X2[:, :, 1:W + 1], 2.0, X2[:, :, 0:W], Alu.mult, Alu.add
            )
            nc.vector.tensor_add(gy[:, :, :], gy[:, :, :], X2[:, :, 2:W + 2])

            # mag = sqrt(gx^2 + gy^2 + 1e-6) into a W-padded buffer.
            # The min-max normalized map m is only consumed through LINEAR
            # functionals (3x3 sums, quadrant sums), so normalization is
            # folded into scalar thresholds; m is never materialized.
            MAGP = maps.tile([P, A, W + 2], f32, tag="MP")
            nc.vector.memset(MAGP[:, :, :], 0.0)
            mag = MAGP[:, :, 1:W + 1]
            nc.vector.tensor_mul(gx[:, :, :], gx[:, :, :], gx[:, :, :])
            nc.vector.tensor_mul(gy[:, :, :], gy[:, :, :], gy[:, :, :])
            nc.vector.tensor_add(mag, gx[:, :, :], gy[:, :, :])
            nc.vector.tensor_scalar_add(mag, mag, 1e-6)
            nc.scalar.activation(mag, mag, Act.Sqrt)

            # min/max over image -> per-partition bcast scalars
            mn = smalls.tile([P, 1], f32, tag="mn")
            mx = smalls.tile([P, 1], f32, tag="mx")
            nc.vector.tensor_reduce(mx[:, :], mag, mybir.AxisListType.XY, Alu.max)
            nc.vector.tensor_reduce(mn[:, :], mag, mybir.AxisListType.XY, Alu.min)
            nc.vector.tensor_scalar_mul(mn[:, :], mn[:, :], -1.0)
            nc.gpsimd.partition_all_reduce(mx[:, :], mx[:, :], P, bass_isa.ReduceOp.max)
            nc.gpsimd.partition_all_reduce(mn[:, :], mn[:, :], P, bass_isa.ReduceOp.max)
            nc.vector.tensor_scalar_mul(mn[:, :], mn[:, :], -1.0)  # true min
            rngp = smalls.tile([P, 1], f32, tag="rngv")  # rng + eps
            nc.vector.tensor_sub(rngp[:, :], mx[:, :], mn[:, :])
            nc.vector.tensor_scalar_add(rngp[:, :], rngp[:, :], 1e-6)

            # ---------------- adj / base / steps ----------------
            # SM9 = 3x3 sum of mag (concurrent with the min/max chain);
            # then R1 = mn*NV - SM9 and adj = (R1 < -HI*rngp) - (R1 > -LO*rngp)
            vfilter(X1, MAGP, D0f[:, :], A01[:, :], A10[:, :], D0f[:, :])
            SM9 = maps.tile([P, A, W], f32, tag="gx")
            nc.vector.tensor_add(SM9[:, :, :], X1[:, :, 0:W], X1[:, :, 1:W + 1])
            nc.vector.tensor_add(SM9[:, :, :], SM9[:, :, :], X1[:, :, 2:W + 2])
            nc.vector.scalar_tensor_tensor(
                SM9[:, :, :], NV[:, :, :], mn[:, :], SM9[:, :, :],
                Alu.mult, Alu.subtract,
            )
            thrH = smalls.tile([P, 1], f32, tag="thrH")
            thrL = smalls.tile([P, 1], f32, tag="thrL")
            nc.vector.tensor_scalar_mul(thrH[:, :], rngp[:, :], -ADJ_HI)
            nc.vector.tensor_scalar_mul(thrL[:, :], rngp[:, :], -ADJ_LO)
            adj = maps.tile([P, A, W], f32, tag="gy")
            t2m = maps.tile([P, A, W], f32, tag="acc")
            nc.vector.tensor_scalar(adj[:, :, :], SM9[:, :, :], thrH[:, :], None, Alu.is_lt)
            nc.vector.tensor_scalar(t2m[:, :, :], SM9[:, :, :], thrL[:, :], None, Alu.is_gt)
            nc.vector.tensor_sub(adj[:, :, :], adj[:, :, :], t2m[:, :, :])

            # quadrant sums of mag -> base (thresholds absorb normalization)
            qp = smalls.tile([P, 4], f32, tag="qp")
            nc.vector.memset(qp[:, :], 0.0)
            for qh in range(2):
                for wh in range(2):
                    col = 2 * qh + wh
                    nc.vector.tensor_reduce(
                        qp[qh * 64:(qh + 1) * 64, col:col + 1],
                        MAGP[qh * 64:(qh + 1) * 64, :, 1 + 128 * wh:1 + 128 * (wh + 1)],
                        mybir.AxisListType.XY, Alu.add,
                    )
            nc.gpsimd.partition_all_reduce(qp[:, :], qp[:, :], P, bass_isa.ReduceOp.add)
            mn16 = smalls.tile([P, 1], f32, tag="mn16")
            nc.vector.tensor_scalar_mul(mn16[:, :], mn[:, :], 16384.0)
            base = smalls.tile([P, 4], f32, tag="base")
            bt = smalls.tile([P, 4], f32, tag="bt")
            rhsk = smalls.tile([P, 1], f32, tag="rhsk")
            nc.vector.memset(base[:, :], 1.0)
            for thr in BASE_THR:
                ck = (thr + 2.5) * 16384.0 / 5.0
                nc.vector.scalar_tensor_tensor(
                    rhsk[:, :], rngp[:, :], ck, mn16[:, :], Alu.mult, Alu.add
                )
                nc.vector.tensor_scalar(bt[:, :], qp[:, :], rhsk[:, :], None, Alu.is_lt)
                nc.vector.tensor_add(base[:, :], base[:, :], bt[:, :])

            # steps_d = clip(adj + base_d, 1, 5), cast to bf16 scan layouts
            SS = scan.tile([P, RG, W], bf16, tag="SS")  # rows x positions
            SD = []
            for d in range(4):
                sd = maps.tile([P, A, W], f32, tag=f"SD{d}")
                nc.vector.tensor_scalar(
                    sd[:, :, :], adj[:, :, :], base[:, d:d + 1], 1.0, Alu.add, Alu.max
                )
                nc.vector.tensor_scalar_min(sd[:, :, :], sd[:, :, :], 5.0)
                SD.append(sd)

            # d0: rows L->R (direct); d1: rows R->L (flip W)
            nc.vector.tensor_copy(SS[:, 0:2, :], SD[0][:, :, :])
            nc.vector.tensor_copy(SS[:, 2:4, :], SD[1][:, :, ::-1])
            # d2/d3: columns; transpose via 128x128 DMA transposes (bf16)
            SD2b = maps.tile([P, A, W], bf16, tag="SD2b")
            SD3b = maps.tile([P, A, W], bf16, tag="SD3b")
            nc.vector.tensor_copy(SD2b[:, :, :], SD[2][:, :, :])
            nc.vector.tensor_copy(SD3b[:, :, :], SD[3][:, :, :])
            # transpose 128x128 blocks into contiguous temps [p, wh, i, q],
            # then one strided DVE copy interleaves pos = 2q + i.
            TT2 = maps.tile([P, 2, 2, 128], bf16, tag="TT2")
            TT3 = maps.tile([P, 2, 2, 128], bf16, tag="TT3")
            for i in range(2):
                for wh in range(2):
                    pe_transpose(TT2[:, wh, i, :], SD2b[:, i, wh * 128:(wh + 1) * 128])
                    pe_transpose(TT3[:, wh, i, :], SD3b[:, i, wh * 128:(wh + 1) * 128])
            nc.vector.tensor_copy(
                SS[:, 4:6, :].rearrange("p r (q i) -> p r q i", i=2),
                TT2[:, :, :, :].transpose([0, 1, 3, 2]),
            )
            # d3: flip pos: 255-(2q+i) = 2*(127-q) + (1-i) -> reverse q and i
            nc.vector.tensor_copy(
                SS[:, 6:8, :].rearrange("p r (q i) -> p r q i", i=2),
                TT3[:, :, ::-1, ::-1].transpose([0, 1, 3, 2]),
            )

            # ---------------- block-automaton scan ----------------
            SSr = SS[:, :, :].rearrange("p r (t j) -> p (r t) j", j=G)  # [P, RT, G]

            # EKDR[i, j] = (S[j - (5-i)] == 5-i), diagonal-aligned, k = 5-i
            EKDR = scan.tile([P, RT, 5, G], bf16, tag="EKDR")
            for k in range(1, 6):
                i = 5 - k
                nc.vector.memset(EKDR[:, :, i, 0:k], 0.0)
                nc.vector.tensor_scalar(
                    EKDR[:, :, i, k:G], SSr[:, :, 0:G - k], float(k), None, Alu.is_equal
                )

            # R[j] = relu(j + S[j] - 16)  (exit offset if jump leaves block)
            Rt = scan.tile([P, RT, G], f32, tag="Rt")
            nc.gpsimd.iota(Rt[:, :, :], pattern=[[0, RT], [1, G]], base=0,
                           channel_multiplier=0,
                           allow_small_or_imprecise_dtypes=True)
            nc.vector.tensor_add(Rt[:, :, :], Rt[:, :, :], SSr)
            nc.vector.tensor_scalar(
                Rt[:, :, :], Rt[:, :, :], -float(G), 0.0, Alu.add, Alu.max
            )

            # phase A: packed window scan. v[j] = sum_e 8^e * visited_e[j]
            # (base-8 so exit offsets 0..4 also pack carry-free, f32-exact)
            v = scan.tile([P, RT, G], f32, tag="v")
            nc.vector.memset(v[:, :, :], 0.0)
            for e in range(5):
                nc.vector.memset(v[:, :, e:e + 1], float(8 ** e))
            for j in range(1, G):
                cnt = min(5, j)
                lo = j - cnt
                tmp = smalls.tile([P, RT, 5], f32, tag="patmp")
                red = smalls.tile([P, RT], f32, tag="pared")
                nc.vector.tensor_mul(
                    tmp[:, :, 0:cnt], v[:, :, lo:j], EKDR[:, :, 5 - j if j < 5 else 0:5, j]
                )
                nc.vector.tensor_reduce(
                    red[:, :], tmp[:, :, 0:cnt], mybir.AxisListType.X, Alu.add
                )
                nc.vector.tensor_add(v[:, :, j], v[:, :, j], red[:, :])

            # packed exit: sum_j v[j]*R[j] = sum_e 8^e * exit_e (carry-free)
            exm = scan.tile([P, RT, G], f32, tag="EKDR")
            exitP = smalls.tile([P, RT], f32, tag="exitP")
            nc.vector.tensor_mul(exm[:, :, :], v[:, :, :], Rt[:, :, :])
            nc.vector.tensor_reduce(
                exitP[:, :], exm[:, :, :], mybir.AxisListType.X, Alu.add
            )
            # unpack exit planes (values 0..4 per plane)
            exitT = scan.tile([P, RG, NB, 5], f32, tag="exitT")
            exitTr = exitT[:, :, :, :].rearrange("p r t e -> p (r t) e")
            bt5 = smalls.tile([P, RT], f32, tag="bt5")
            for e in range(4, -1, -1):
                sc8 = float(8 ** e)
                nc.vector.tensor_scalar(
                    exitTr[:, :, e], exitP[:, :], 1.0 * sc8, None, Alu.is_ge
                )
                for c in (2.0, 3.0, 4.0):
                    nc.vector.tensor_scalar(
                        bt5[:, :], exitP[:, :], c * sc8, None, Alu.is_ge
                    )
                    nc.vector.tensor_add(exitTr[:, :, e], exitTr[:, :, e], bt5[:, :])
                if e > 0:
                    nc.vector.scalar_tensor_tensor(
                        exitP[:, :], exitTr[:, :, e], -sc8, exitP[:, :],
                        Alu.mult, Alu.add,
                    )

            # unpack visited planes: VP[e] contiguous (destroys v)
            VP = scan.tile([P, 5, RT, G], bf16, tag="VP")
            for e in range(4, -1, -1):
                nc.vector.tensor_scalar(
                    VP[:, e, :, :], v[:, :, :], float(8 ** e), None, Alu.is_ge
                )
                if e > 0:
                    nc.vector.scalar_tensor_tensor(
                        v[:, :, :], VP[:, e, :, :], -float(8 ** e), v[:, :, :],
                        Alu.mult, Alu.add,
                    )

            # phase B: chain entry states across blocks
            CONSTE = scan.tile([P, RG, 5, 5], bf16, tag="CONSTE")
            CEi = scan.tile([P, RG, 5, 5], i32, tag="CEi")
            nc.gpsimd.iota(CEi[:, :, :, :], pattern=[[0, RG], [1, 5], [0, 5]], base=0,
                           channel_multiplier=0)
            nc.vector.tensor_copy(CONSTE[:, :, :, :], CEi[:, :, :, :])
            stall = scan.tile([P, RG, NB, 5], f32, tag="stall")
            nc.vector.memset(stall[:, :, :, :], 0.0)
            nc.vector.memset(stall[:, :, 0, 0:1], 1.0)
            for t in range(NB - 1):
                Xt = smalls.tile([P, RG, 5, 5], bf16, tag="Xt")
                nc.vector.tensor_tensor(
                    Xt[:, :, :, :],
                    exitT[:, :, t, :].unsqueeze(2).broadcast_to((P, RG, 5, 5)),
                    CONSTE[:, :, :, :],
                    Alu.is_equal,
                )
                nc.vector.tensor_mul(
                    Xt[:, :, :, :], Xt[:, :, :, :],
                    stall[:, :, t, :].unsqueeze(2).broadcast_to((P, RG, 5, 5)),
                )
                nc.vector.tensor_reduce(
                    stall[:, :, t + 1, :], Xt[:, :, :, :], mybir.AxisListType.X, Alu.add
                )

            # selection: vis[rt, j] = sum_e stall[rt, e] * VP[rt, e, j]
            vis = scan.tile([P, RG, W], bf16, tag="vis")
            visr = vis[:, :, :].rearrange("p r (t j) -> p r t j", j=G)
            stmp = scan.tile([P, RG, NB, G], bf16, tag="EKDR")
            stmp2 = scan.tile([P, RG, NB, G], bf16, tag="stmp2")
            for e in range(5):
                vpe = VP[:, e, :, :].rearrange("p (r t) j -> p r t j", r=RG)
                ste = stall[:, :, :, e].unsqueeze(3).broadcast_to((P, RG, NB, G))
                if e == 0:
                    nc.vector.tensor_mul(visr, vpe, ste)
                elif e == 3:
                    nc.gpsimd.tensor_mul(stmp2[:, :, :, :], vpe, ste)
                elif e == 4:
                    tg = scan.tile([P, RG, NB, G], bf16, tag="stmp3")
                    nc.gpsimd.tensor_mul(tg[:, :, :, :], vpe, ste)
                    nc.gpsimd.tensor_add(stmp2[:, :, :, :], stmp2[:, :, :, :],
                                         tg[:, :, :, :])
                else:
                    nc.vector.tensor_mul(stmp[:, :, :, :], vpe, ste)
                    nc.vector.tensor_add(visr, visr, stmp[:, :, :, :])
            nc.vector.tensor_add(visr, visr, stmp2[:, :, :, :])

            # ---------------- combine directions ----------------
            VT2 = maps.tile([P, A, W], bf16, tag="SD0")
            VT3 = maps.tile([P, A, W], bf16, tag="SD1")
            r3 = maps.tile([P, A, W], bf16, tag="r3")
            nc.vector.tensor_copy(r3[:, :, :], vis[:, 6:8, ::-1])
            VTMP2 = maps.tile([P, 2, 2, 128], bf16, tag="TT2")
            VTMP3 = maps.tile([P, 2, 2, 128], bf16, tag="TT3")
            nc.vector.tensor_copy(
                VTMP2[:, :, :, :],
                vis[:, 4:6, :].rearrange("p r (q i) -> p r q i", i=2).transpose(
                    [0, 1, 3, 2]
                ),
            )
            nc.vector.tensor_copy(
                VTMP3[:, :, :, :],
                r3[:, :, :].rearrange("p r (q i) -> p r q i", i=2).transpose(
                    [0, 1, 3, 2]
                ),
            )
            for i in range(2):
                for wh in range(2):
                    pe_transpose(VT2[:, i, wh * 128:(wh + 1) * 128], VTMP2[:, wh, i, :])
                    pe_transpose(VT3[:, i, wh * 128:(wh + 1) * 128], VTMP3[:, wh, i, :])
            Vm = maps.tile([P, A, W], f32, tag="SD2")
            Vt = maps.tile([P, A, W], f32, tag="SD3")
            nc.vector.tensor_add(Vm[:, :, :], vis[:, 0:2, :], vis[:, 2:4, ::-1])
            nc.vector.tensor_add(Vt[:, :, :], VT2[:, :, :], VT3[:, :, :])
            nc.vector.tensor_add(Vm[:, :, :], Vm[:, :, :], Vt[:, :, :])

            # factor = V / (V + 1e-6)
            fac = maps.tile([P, A, W], f32, tag="gx")
            nc.vector.tensor_scalar(fac[:, :, :], Vm[:, :, :], 0.0, None, Alu.is_gt)

            # ---------------- pass 2: out = x * factor ----------------
            # replicate factor across the channel axis once -> contiguous muls
            facr = scan.tile([P, CC, A, W], f32, tag="EKDR")
            nc.vector.tensor_copy(
                facr[:, :, :, :],
                fac[:, :, :].unsqueeze(1).broadcast_to((P, CC, A, W)),
            )
            for ci in range(NCHUNK):
                xt = stream.tile([P, CC, A, W], f32, tag="xin")
                nc.sync.dma_start(
                    out=xt[:, :, :, :],
                    in_=x[ci * CC:(ci + 1) * CC].rearrange(
                        "c (p i) w -> p c i w", i=2
                    ),
                )
                meng = nc.gpsimd if ci % 3 == 2 else nc.vector
                meng.tensor_mul(xt[:, :, :, :], xt[:, :, :, :], facr[:, :, :, :])
                nc.scalar.dma_start(
                    out=out[ci * CC:(ci + 1) * CC].rearrange(
                        "c (p i) w -> p c i w", i=2
                    ),
                    in_=xt[:, :, :, :],
                )

    nc.compile()
    return nc


def _get_nc():
    if "nc" not in _NC_CACHE:
        _NC_CACHE["nc"] = _build_nc()
    return _NC_CACHE["nc"]


def kernel(x):
    from concourse.bass_utils import run_bass_kernel_spmd

    x = np.ascontiguousarray(np.asarray(x, dtype=np.float32))
    B = x.shape[0]
    nc = _get_nc()
    in_maps = [{"x": np.ascontiguousarray(x[b])} for b in range(B)]
    res = run_bass_kernel_spmd(nc, in_maps, core_ids=list(range(B)))
    return np.stack([res.results[b]["out"] for b in range(B)], axis=0)


# revision 36
# speedup vs baseline: 1.1537x; 1.1537x over previous
"""Trainium2 Bass kernel for nn_AJSSMamba (adaptive directional scan).

Self-contained: shards batch 8 across 8 NeuronCores (1 sample/core),
computes the full module on-device, gathers outputs on host.

Per-core pipeline (sample x [96,256,256] f32):
  1. channel mean (streamed)         -> xg [256,256]
  2. sobel complexity + minmax norm  -> m
  3. local 3x3 avg -> adj; quadrant means -> base; steps[d] = clip(base_d+adj,1,5)
  4. four directional adaptive-step traversals via a block-automaton scan:
     256 positions = 16 blocks x 16; 5 entry offsets packed as base-2 bit
     planes of one bf16 number (out-degree-1 traversal => no carries);
     cross-block chaining via 5-state one-hot automaton.
  5. V = sum of 4 visit masks; factor = V/(V+1e-6); out = x * factor.
"""

import sys

sys.path.insert(0, "/opt/trn_rl_repo")

import math

import numpy as np

P = 128          # partitions
C = 96           # channels
H = 256
W = 256
A = 2            # H // P
CC = 8           # channels per stream chunk
NCHUNK = C // CC
G = 16           # scan block size
NB = 16          # number of blocks (G*NB == W)
RG = 8           # row groups: 4 directions x (256 rows / 128)
RT = RG * NB     # collapsed (rg, block) groups

LN3 = math.log(3.0)
ADJ_HI = 9.0 * (2.5 + LN3) / 5.0   # lc9 threshold for adj=+1
ADJ_LO = 9.0 * (2.5 - LN3) / 5.0   # lc9 threshold for adj=-1
# base = 1 + sum_k [pre < thr_k], pre = qsum*(5/16384) - 2.5
BASE_THR = [math.log(7.0), math.log(5.0 / 3.0), -math.log(5.0 / 3.0), -math.log(7.0)]

_NC_CACHE = {}


def _build_nc():
    from concourse import bacc, mybir
    from concourse.tile import TileContext
    from concourse import bass_isa

    f32 = mybir.dt.float32
    bf16 = mybir.dt.bfloat16
    i32 = mybir.dt.int32
    Alu = mybir.AluOpType
    Act = mybir.ActivationFunctionType

    nc = bacc.Bacc(None, target_bir_lowering=False, debug=False)
    x = nc.declare_dram_parameter("x", [C, H, W], f32, isOutput=False)
    out = nc.declare_dram_parameter("out", [C, H, W], f32, isOutput=True)

    with TileContext(nc) as tc:
        with (
            tc.tile_pool(name="stream", bufs=5) as stream,
            tc.tile_pool(name="maps", bufs=1) as maps,
            tc.tile_pool(name="scan", bufs=1) as scan,
            tc.tile_pool(name="smalls", bufs=2) as smalls,
            tc.tile_pool(name="consts", bufs=1) as consts,
            tc.tile_pool(name="psum", bufs=4, space="PSUM") as psum,
        ):
            # ---------------- constants (banded matrices for vertical
            # filters along h = 2p+i, and the PE-transpose identity) -----
            ITi = consts.tile([P, 128], i32, tag="ITi")
            nc.gpsimd.iota(ITi[:, :], pattern=[[-1, 128]], base=0,
                           channel_multiplier=1)  # value = k - m
            D0f = consts.tile([P, 128], f32, tag="D0f")
            Dm1 = consts.tile([P, 128], f32, tag="Dm1")
            Dp1 = consts.tile([P, 128], f32, tag="Dp1")
            D2f = consts.tile([P, 128], f32, tag="D2f")
            A01 = consts.tile([P, 128], f32, tag="A01")  # D0 + Dm1
            A10 = consts.tile([P, 128], f32, tag="A10")  # D0 + Dp1
            M01 = consts.tile([P, 128], f32, tag="M01")  # Dm1 - D0
            M10 = consts.tile([P, 128], f32, tag="M10")  # D0 - Dp1
            IDb = consts.tile([P, 128], bf16, tag="IDb")
            nc.vector.tensor_scalar(D0f[:, :], ITi[:, :], 0.0, None, Alu.is_equal)
            nc.vector.tensor_scalar(Dm1[:, :], ITi[:, :], -1.0, None, Alu.is_equal)
            nc.vector.tensor_scalar(Dp1[:, :], ITi[:, :], 1.0, None, Alu.is_equal)
            nc.vector.tensor_scalar_mul(D2f[:, :], D0f[:, :], 2.0)
            nc.vector.tensor_add(A01[:, :], D0f[:, :], Dm1[:, :])
            nc.vector.tensor_add(A10[:, :], D0f[:, :], Dp1[:, :])
            nc.vector.tensor_sub(M01[:, :], Dm1[:, :], D0f[:, :])
            nc.vector.tensor_sub(M10[:, :], D0f[:, :], Dp1[:, :])
            nc.vector.tensor_scalar(IDb[:, :], ITi[:, :], 0.0, None, Alu.is_equal)

            def vfilter(dst, src, lhs0_for_i0, lhs1_for_i0, lhs0_for_i1,
                        lhs1_for_i1):
                # dst[:, i, :] = banded vertical filter of src (padded W+2)
                for i in range(2):
                    l0 = lhs0_for_i0 if i == 0 else lhs0_for_i1
                    l1 = lhs1_for_i0 if i == 0 else lhs1_for_i1
                    ps = psum.tile([P, W + 2], f32, tag="ps")
                    if l0 is not None and l1 is not None:
                        nc.tensor.matmul(ps[:, :], l0, src[:, 0, :],
                                         start=True, stop=False)
                        nc.tensor.matmul(ps[:, :], l1, src[:, 1, :],
                                         start=False, stop=True)
                    elif l0 is not None:
                        nc.tensor.matmul(ps[:, :], l0, src[:, 0, :],
                                         start=True, stop=True)
                    else:
                        nc.tensor.matmul(ps[:, :], l1, src[:, 1, :],
                                         start=True, stop=True)
                    nc.scalar.copy(dst[:, i, :], ps[:, :])

            def pe_transpose(dst, src):
                # dst = src.T for [128,128] bf16 tiles via PE
                tp = psum.tile([P, 128], bf16, tag="tp")
                nc.tensor.transpose(tp[:, :], src, IDb[:, :])
                nc.scalar.copy(dst, tp[:, :])

            # NV[h,w] = number of in-bounds 3x3 neighbors (9/6/4) -- used to
            # fold the min-max normalization into scalar thresholds
            ONESP = consts.tile([P, A, W + 2], f32, tag="ONESP")
            nc.vector.memset(ONESP[:, :, :], 0.0)
            nc.vector.memset(ONESP[:, :, 1:W + 1], 1.0)
            XV = consts.tile([P, A, W + 2], f32, tag="XV")
            vfilter(XV, ONESP, D0f[:, :], A01[:, :], A10[:, :], D0f[:, :])
            NV = consts.tile([P, A, W], f32, tag="NV")
            nc.vector.tensor_add(NV[:, :, :], XV[:, :, 0:W], XV[:, :, 1:W + 1])
            nc.vector.tensor_add(NV[:, :, :], NV[:, :, :], XV[:, :, 2:W + 2])
            # ---------------- pass 1: channel mean ----------------
            acc = maps.tile([P, A, W], f32, tag="acc")
            acc2 = maps.tile([P, A, W], f32, tag="acc2")
            for ci in range(NCHUNK):
                xt = stream.tile([P, CC, A, W], f32, tag="xin")
                ldeng = nc.sync if ci % 2 == 0 else nc.scalar
                ldeng.dma_start(
                    out=xt[:, :, :, :],
                    in_=x[ci * CC:(ci + 1) * CC].rearrange(
                        "c (p i) w -> p c i w", i=2
                    ),
                )
                # tree-reduce the 8 channels: 8 -> 4 -> 2 -> 1
                # alternate chunks between DVE and GpSimd (both 1x fp32);
                # per-engine partial accumulators merge at the end.
                on_gps = (ci % 3 == 2)
                eng = nc.gpsimd if on_gps else nc.vector
                accx = acc2 if on_gps else acc
                first = (ci == 2) if on_gps else (ci == 0)
                if on_gps:
                    t4 = scan.tile([P, 4, A, W], f32, tag="VP")
                    t2 = scan.tile([P, 2, A, W], f32, tag="vis")
                else:
                    t4 = scan.tile([P, 4, A, W], f32, tag="Rt")
                    t2 = scan.tile([P, 2, A, W], f32, tag="SS")
                eng.tensor_add(t4[:, :, :, :], xt[:, 0:4, :, :], xt[:, 4:8, :, :])
                eng.tensor_add(t2[:, :, :, :], t4[:, 0:2, :, :], t4[:, 2:4, :, :])
                if first:
                    eng.tensor_add(accx[:, :, :], t2[:, 0, :, :], t2[:, 1, :, :])
                else:
                    eng.tensor_add(t2[:, 0, :, :], t2[:, 0, :, :], t2[:, 1, :, :])
                    eng.tensor_add(accx[:, :, :], accx[:, :, :], t2[:, 0, :, :])
            nc.vector.tensor_add(acc[:, :, :], acc[:, :, :], acc2[:, :, :])

            # xg = acc / 96, into W-padded buffer MP [P, A, W+2]
            MP = maps.tile([P, A, W + 2], f32, tag="MP")
            nc.vector.memset(MP[:, :, :], 0.0)
            nc.vector.tensor_scalar_mul(MP[:, :, 1:W + 1], acc[:, :, :], 1.0 / C)

            # ---------------- sobel ----------------
            # vertical filters along h via banded matmuls on the (idle) PE:
            # X1 = [1,2,1]_v xg (padded), X2 = [1,0,-1]_v xg
            X1 = maps.tile([P, A, W + 2], f32, tag="X1")
            X2 = maps.tile([P, A, W + 2], f32, tag="X2")
            vfilter(X1, MP, D2f[:, :], A01[:, :], A10[:, :], D2f[:, :])
            vfilter(X2, MP, None, M01[:, :], M10[:, :], None)

            # horizontal: gx = X1[w-1]-X1[w+1]; gy = X2[w-1]+2*X2[w]+X2[w+1]
            gx = maps.tile([P, A, W], f32, tag="gx")
            gy = maps.tile([P, A, W], f32, tag="gy")
            nc.vector.tensor_sub(gx[:, :, :], X1[:, :, 0:W], X1[:, :, 2:W + 2])
            nc.vector.scalar_tensor_tensor(
                gy[:, :, :], X2[:, :, 1:W + 1], 2.0, X2[:, :, 0:W], Alu.mult, Alu.add
            )
            nc.vector.tensor_add(gy[:, :, :], gy[:, :, :], X2[:, :, 2:W + 2])

            # mag = sqrt(gx^2 + gy^2 + 1e-6) into a W-padded buffer.
            # The min-max normalized map m is only consumed through LINEAR
            # functionals (3x3 sums, quadrant sums), so normalization is
            # folded into scalar thresholds; m is never materialized.
            MAGP = maps.tile([P, A, W + 2], f32, tag="MP")
            nc.vector.memset(MAGP[:, :, :], 0.0)
            mag = MAGP[:, :, 1:W + 1]
            nc.vector.tensor_mul(gx[:, :, :], gx[:, :, :], gx[:, :, :])
            nc.vector.tensor_mul(gy[:, :, :], gy[:, :, :], gy[:, :, :])
            nc.vector.tensor_add(mag, gx[:, :, :], gy[:, :, :])
            nc.vector.tensor_scalar_add(mag, mag, 1e-6)
            nc.scalar.activation(mag, mag, Act.Sqrt)

            # min/max over image -> per-partition bcast scalars
            mn = smalls.tile([P, 1], f32, tag="mn")
            mx = smalls.tile([P, 1], f32, tag="mx")
            nc.vector.tensor_reduce(mx[:, :], mag, mybir.AxisListType.XY, Alu.max)
            nc.vector.tensor_reduce(mn[:, :], mag, mybir.AxisListType.XY, Alu.min)
            nc.vector.tensor_scalar_mul(mn[:, :], mn[:, :], -1.0)
            nc.gpsimd.partition_all_reduce(mx[:, :], mx[:, :], P, bass_isa.ReduceOp.max)
            nc.gpsimd.partition_all_reduce(mn[:, :], mn[:, :], P, bass_isa.ReduceOp.max)
            nc.vector.tensor_scalar_mul(mn[:, :], mn[:, :], -1.0)  # true min
            rngp = smalls.tile([P, 1], f32, tag="rngv")  # rng + eps
            nc.vector.tensor_sub(rngp[:, :], mx[:, :], mn[:, :])
            nc.vector.tensor_scalar_add(rngp[:, :], rngp[:, :], 1e-6)

            # ---------------- adj / base / steps ----------------
            # SM9 = 3x3 sum of mag (concurrent with the min/max chain);
            # then R1 = mn*NV - SM9 and adj = (R1 < -HI*rngp) - (R1 > -LO*rngp)
            vfilter(X1, MAGP, D0f[:, :], A01[:, :], A10[:, :], D0f[:, :])
            SM9 = maps.tile([P, A, W], f32, tag="gx")
            nc.vector.tensor_add(SM9[:, :, :], X1[:, :, 0:W], X1[:, :, 1:W + 1])
            nc.vector.tensor_add(SM9[:, :, :], SM9[:, :, :], X1[:, :, 2:W + 2])
            nc.vector.scalar_tensor_tensor(
                SM9[:, :, :], NV[:, :, :], mn[:, :], SM9[:, :, :],
                Alu.mult, Alu.subtract,
            )
            thrH = smalls.tile([P, 1], f32, tag="thrH")
            thrL = smalls.tile([P, 1], f32, tag="thrL")
            nc.vector.tensor_scalar_mul(thrH[:, :], rngp[:, :], -ADJ_HI)
            nc.vector.tensor_scalar_mul(thrL[:, :], rngp[:, :], -ADJ_LO)
            adj = maps.tile([P, A, W], f32, tag="gy")
            t2m = maps.tile([P, A, W], f32, tag="acc")
            nc.vector.tensor_scalar(adj[:, :, :], SM9[:, :, :], thrH[:, :], None, Alu.is_lt)
            nc.vector.tensor_scalar(t2m[:, :, :], SM9[:, :, :], thrL[:, :], None, Alu.is_gt)
            nc.vector.tensor_sub(adj[:, :, :], adj[:, :, :], t2m[:, :, :])

            # quadrant sums of mag -> base (thresholds absorb normalization)
            qp = smalls.tile([P, 4], f32, tag="qp")
            nc.vector.memset(qp[:, :], 0.0)
            for qh in range(2):
                for wh in range(2):
                    col = 2 * qh + wh
                    nc.vector.tensor_reduce(
                        qp[qh * 64:(qh + 1) * 64, col:col + 1],
                        MAGP[qh * 64:(qh + 1) * 64, :, 1 + 128 * wh:1 + 128 * (wh + 1)],
                        mybir.AxisListType.XY, Alu.add,
                    )
            nc.gpsimd.partition_all_reduce(qp[:, :], qp[:, :], P, bass_isa.ReduceOp.add)
            mn16 = smalls.tile([P, 1], f32, tag="mn16")
            nc.vector.tensor_scalar_mul(mn16[:, :], mn[:, :], 16384.0)
            base = smalls.tile([P, 4], f32, tag="base")
            bt = smalls.tile([P, 4], f32, tag="bt")
            rhsk = smalls.tile([P, 1], f32, tag="rhsk")
            nc.vector.memset(base[:, :], 1.0)
            for thr in BASE_THR:
                ck = (thr + 2.5) * 16384.0 / 5.0
                nc.vector.scalar_tensor_tensor(
                    rhsk[:, :], rngp[:, :], ck, mn16[:, :], Alu.mult, Alu.add
                )
                nc.vector.tensor_scalar(bt[:, :], qp[:, :], rhsk[:, :], None, Alu.is_lt)
                nc.vector.tensor_add(base[:, :], base[:, :], bt[:, :])

            # steps_d = clip(adj + base_d, 1, 5), cast to bf16 scan layouts
            SS = scan.tile([P, RG, W], bf16, tag="SS")  # rows x positions
            SD = []
            for d in range(4):
                sd = maps.tile([P, A, W], f32, tag=f"SD{d}")
                nc.vector.tensor_scalar(
                    sd[:, :, :], adj[:, :, :], base[:, d:d + 1], 1.0, Alu.add, Alu.max
                )
                nc.vector.tensor_scalar_min(sd[:, :, :], sd[:, :, :], 5.0)
                SD.append(sd)

            # d0: rows L->R (direct); d1: rows R->L (flip W)
            nc.vector.tensor_copy(SS[:, 0:2, :], SD[0][:, :, :])
            nc.vector.tensor_copy(SS[:, 2:4, :], SD[1][:, :, ::-1])
            # d2/d3: columns; transpose via 128x128 DMA transposes (bf16)
            SD2b = maps.tile([P, A, W], bf16, tag="SD2b")
            SD3b = maps.tile([P, A, W], bf16, tag="SD3b")
            nc.vector.tensor_copy(SD2b[:, :, :], SD[2][:, :, :])
            nc.vector.tensor_copy(SD3b[:, :, :], SD[3][:, :, :])
            # transpose 128x128 blocks into contiguous temps [p, wh, i, q],
            # then one strided DVE copy interleaves pos = 2q + i.
            TT2 = maps.tile([P, 2, 2, 128], bf16, tag="TT2")
            TT3 = maps.tile([P, 2, 2, 128], bf16, tag="TT3")
            for i in range(2):
                for wh in range(2):
                    pe_transpose(TT2[:, wh, i, :], SD2b[:, i, wh * 128:(wh + 1) * 128])
                    pe_transpose(TT3[:, wh, i, :], SD3b[:, i, wh * 128:(wh + 1) * 128])
            nc.vector.tensor_copy(
                SS[:, 4:6, :].rearrange("p r (q i) -> p r q i", i=2),
                TT2[:, :, :, :].transpose([0, 1, 3, 2]),
            )
            # d3: flip pos: 255-(2q+i) = 2*(127-q) + (1-i) -> reverse q and i
            nc.vector.tensor_copy(
                SS[:, 6:8, :].rearrange("p r (q i) -> p r q i", i=2),
                TT3[:, :, ::-1, ::-1].transpose([0, 1, 3, 2]),
            )

            # ---------------- block-automaton scan ----------------
            SSr = SS[:, :, :].rearrange("p r (t j) -> p (r t) j", j=G)  # [P, RT, G]

            # EKDR[i, j] = (S[j - (5-i)] == 5-i), diagonal-aligned, k = 5-i
            EKDR = scan.tile([P, RT, 5, G], bf16, tag="EKDR")
            for k in range(1, 6):
                i = 5 - k
                nc.vector.memset(EKDR[:, :, i, 0:k], 0.0)
                nc.vector.tensor_scalar(
                    EKDR[:, :, i, k:G], SSr[:, :, 0:G - k], float(k), None, Alu.is_equal
                )

            # R[j] = relu(j + S[j] - 16)  (exit offset if jump leaves block)
            Rt = scan.tile([P, RT, G], f32, tag="Rt")
            nc.gpsimd.iota(Rt[:, :, :], pattern=[[0, RT], [1, G]], base=0,
                           channel_multiplier=0,
                           allow_small_or_imprecise_dtypes=True)
            nc.vector.tensor_add(Rt[:, :, :], Rt[:, :, :], SSr)
            nc.vector.tensor_scalar(
                Rt[:, :, :], Rt[:, :, :], -float(G), 0.0, Alu.add, Alu.max
            )

            # phase A: packed window scan. v[j] = sum_e 8^e * visited_e[j]
            # (base-8 so exit offsets 0..4 also pack carry-free, f32-exact)
            v = scan.tile([P, RT, G], f32, tag="v")
            nc.vector.memset(v[:, :, :], 0.0)
            for e in range(5):
                nc.vector.memset(v[:, :, e:e + 1], float(8 ** e))
            for j in range(1, G):
                cnt = min(5, j)
                lo = j - cnt
                tmp = smalls.tile([P, RT, 5], f32, tag="patmp")
                red = smalls.tile([P, RT], f32, tag="pared")
                nc.vector.tensor_mul(
                    tmp[:, :, 0:cnt], v[:, :, lo:j], EKDR[:, :, 5 - j if j < 5 else 0:5, j]
                )
                nc.vector.tensor_reduce(
                    red[:, :], tmp[:, :, 0:cnt], mybir.AxisListType.X, Alu.add
                )
                nc.vector.tensor_add(v[:, :, j], v[:, :, j], red[:, :])

            # packed exit: sum_j v[j]*R[j] = sum_e 8^e * exit_e (carry-free)
            exm = scan.tile([P, RT, G], f32, tag="EKDR")
            exitP = smalls.tile([P, RT], f32, tag="exitP")
            nc.vector.tensor_mul(exm[:, :, :], v[:, :, :], Rt[:, :, :])
            nc.vector.tensor_reduce(
                exitP[:, :], exm[:, :, :], mybir.AxisListType.X, Alu.add
            )
            # unpack exit planes (values 0..4 per plane)
            exitT = scan.tile([P, RG, NB, 5], f32, tag="exitT")
            exitTr = exitT[:, :, :, :].rearrange("p r t e -> p (r t) e")
            bt5 = smalls.tile([P, RT], f32, tag="bt5")
            for e in range(4, -1, -1):
                sc8 = float(8 ** e)
                nc.vector.tensor_scalar(
                    exitTr[:, :, e], exitP[:, :], 1.0 * sc8, None, Alu.is_ge
                )
                for c in (2.0, 3.0, 4.0):
                    nc.vector.tensor_scalar(
                        bt5[:, :], exitP[:, :], c * sc8, None, Alu.is_ge
                    )
                    nc.vector.tensor_add(exitTr[:, :, e], exitTr[:, :, e], bt5[:, :])
                if e > 0:
                    nc.vector.scalar_tensor_tensor(
                        exitP[:, :], exitTr[:, :, e], -sc8, exitP[:, :],
                        Alu.mult, Alu.add,
                    )

            # unpack visited planes: VP[e] contiguous (destroys v)
            VP = scan.tile([P, 5, RT, G], bf16, tag="VP")
            for e in range(4, -1, -1):
                nc.vector.tensor_scalar(
                    VP[:, e, :, :], v[:, :, :], float(8 ** e), None, Alu.is_ge
                )
                if e > 0:
                    nc.vector.scalar_tensor_tensor(
                        v[:, :, :], VP[:, e, :, :], -float(8 ** e), v[:, :, :],
                        Alu.mult, Alu.add,
                    )

            # phase B: chain entry states across blocks
            CONSTE = scan.tile([P, RG, 5, 5], bf16, tag="CONSTE")
            CEi = scan.tile([P, RG, 5, 5], i32, tag="CEi")
            nc.gpsimd.iota(CEi[:, :, :, :], pattern=[[0, RG], [1, 5], [0, 5]], base=0,
                           channel_multiplier=0)
            nc.vector.tensor_copy(CONSTE[:, :, :, :], CEi[:, :, :, :])
            stall = scan.tile([P, RG, NB, 5], f32, tag="stall")
            nc.vector.memset(stall[:, :, :, :], 0.0)
            nc.vector.memset(stall[:, :, 0, 0:1], 1.0)
            for t in range(NB - 1):
                Xt = smalls.tile([P, RG, 5, 5], bf16, tag="Xt")
                nc.vector.tensor_tensor(
                    Xt[:, :, :, :],
                    exitT[:, :, t, :].unsqueeze(2).broadcast_to((P, RG, 5, 5)),
                    CONSTE[:, :, :, :],
                    Alu.is_equal,
                )
                nc.vector.tensor_mul(
                    Xt[:, :, :, :], Xt[:, :, :, :],
                    stall[:, :, t, :].unsqueeze(2).broadcast_to((P, RG, 5, 5)),
                )
                nc.vector.tensor_reduce(
                    stall[:, :, t + 1, :], Xt[:, :, :, :], mybir.AxisListType.X, Alu.add
                )

            # selection: vis[rt, j] = sum_e stall[rt, e] * VP[rt, e, j]
            vis = scan.tile([P, RG, W], bf16, tag="vis")
            visr = vis[:, :, :].rearrange("p r (t j) -> p r t j", j=G)
            stmp = scan.tile([P, RG, NB, G], bf16, tag="EKDR")
            for e in range(5):
                vpe = VP[:, e, :, :].rearrange("p (r t) j -> p r t j", r=RG)
                ste = stall[:, :, :, e].unsqueeze(3).broadcast_to((P, RG, NB, G))
                if e == 0:
                    nc.vector.tensor_mul(visr, vpe, ste)
                else:
                    nc.vector.tensor_mul(stmp[:, :, :, :], vpe, ste)
                    nc.vector.tensor_add(visr, visr, stmp[:, :, :, :])

            # ---------------- combine directions ----------------
            VT2 = maps.tile([P, A, W], bf16, tag="SD0")
            VT3 = maps.tile([P, A, W], bf16, tag="SD1")
            r3 = maps.tile([P, A, W], bf16, tag="r3")
            nc.vector.tensor_copy(r3[:, :, :], vis[:, 6:8, ::-1])
            VTMP2 = maps.tile([P, 2, 2, 128], bf16, tag="TT2")
            VTMP3 = maps.tile([P, 2, 2, 128], bf16, tag="TT3")
            nc.vector.tensor_copy(
                VTMP2[:, :, :, :],
                vis[:, 4:6, :].rearrange("p r (q i) -> p r q i", i=2).transpose(
                    [0, 1, 3, 2]
                ),
            )
            nc.vector.tensor_copy(
                VTMP3[:, :, :, :],
                r3[:, :, :].rearrange("p r (q i) -> p r q i", i=2).transpose(
                    [0, 1, 3, 2]
                ),
            )
            for i in range(2):
                for wh in range(2):
                    pe_transpose(VT2[:, i, wh * 128:(wh + 1) * 128], VTMP2[:, wh, i, :])
                    pe_transpose(VT3[:, i, wh * 128:(wh + 1) * 128], VTMP3[:, wh, i, :])
            Vm = maps.tile([P, A, W], f32, tag="SD2")
            Vt = maps.tile([P, A, W], f32, tag="SD3")
            nc.vector.tensor_add(Vm[:, :, :], vis[:, 0:2, :], vis[:, 2:4, ::-1])
            nc.vector.tensor_add(Vt[:, :, :], VT2[:, :, :], VT3[:, :, :])
            nc.vector.tensor_add(Vm[:, :, :], Vm[:, :, :], Vt[:, :, :])

            # factor = V / (V + 1e-6)
            fac = maps.tile([P, A, W], f32, tag="gx")
            nc.vector.tensor_scalar(fac[:, :, :], Vm[:, :, :], 0.0, None, Alu.is_gt)

            # ---------------- pass 2: out = x * factor ----------------
            # replicate factor across the channel axis once -> contiguous muls
            facr = scan.tile([P, CC, A, W], f32, tag="EKDR")
            nc.vector.tensor_copy(
                facr[:, :, :, :],
                fac[:, :, :].unsqueeze(1).broadcast_to((P, CC, A, W)),
            )
            for ci in range(NCHUNK):
                xt = stream.tile([P, CC, A, W], f32, tag="xin")
                nc.sync.dma_start(
                    out=xt[:, :, :, :],
                    in_=x[ci * CC:(ci + 1) * CC].rearrange(
                        "c (p i) w -> p c i w", i=2
                    ),
                )
                meng = nc.gpsimd if ci % 3 == 2 else nc.vector
                meng.tensor_mul(xt[:, :, :, :], xt[:, :, :, :], facr[:, :, :, :])
                nc.scalar.dma_start(
                    out=out[ci * CC:(ci + 1) * CC].rearrange(
                        "c (p i) w -> p c i w", i=2
                    ),
                    in_=xt[:, :, :, :],
                )

    nc.compile()
    return nc


def _get_nc():
    if "nc" not in _NC_CACHE:
        _NC_CACHE["nc"] = _build_nc()
    return _NC_CACHE["nc"]


def kernel(x):
    from concourse.bass_utils import run_bass_kernel_spmd

    x = np.ascontiguousarray(np.asarray(x, dtype=np.float32))
    B = x.shape[0]
    nc = _get_nc()
    in_maps = [{"x": np.ascontiguousarray(x[b])} for b in range(B)]
    res = run_bass_kernel_spmd(nc, in_maps, core_ids=list(range(B)))
    return np.stack([res.results[b]["out"] for b in range(B)], axis=0)
